# revision 1
# baseline (speedup 1.0000x reference)
"""Trainium2 Bass kernel for nn_F_local_72335839200042.

MLP (300000x992 @ 992x32) -> KAN layer (32->6, G=8 K=4 B-splines) -> KAN layer (6->1).

Strategy (pure data parallel over 8 NeuronCores, N sharded):
  - Host: pad N to 8*74*512, shard, split x into fp16 hi/lo pair, transpose to
    [992, Npc] so the 992-contraction lands on SBUF partitions.
  - hidden4 = -4*(x @ mlp_w) replicated 4x across 128 partitions, via 24 fp16
    matmuls (hh, hl, lh passes; ll term ~2^-21 dropped) accumulating in PSUM.
  - B-spline basis: B_b(h) = (a^4 - 5 b^4 + 10 c^4)/24 with atoms
    ((shift_pi - u)_+)^4, u = |4h + 5.5 - b|; u via ACT Abs (per-partition
    bias), atoms+combine via 3 chained custom DVE ops per feature chunk,
    keeping small-range basis values in f32 (exact).
  - Layer contractions on PE in fp32 (4 cyc/row); silu via ACT Swish.
  - Layer 1 basis on 72 = 12 basis x 6 hidden features; its silu rides the
    same PE pass via 6 extra replica rows in the P1 accumulator (rows 72:78).
"""

import sys

for _p in ("/opt/trn_rl_repo",):
    if _p not in sys.path:
        sys.path.insert(0, _p)

import numpy as np

G = 8
K = 4
NB = G + K          # 12 basis fns per input dim
NBLK = 512          # rows per block (one PSUM bank of fp32)
BLOCKS = 74         # blocks per core
NPC = NBLK * BLOCKS  # rows per core (padded)
NCORES = 8
D_IN = 992
H = 32
H1 = 6
DCH = 124           # contraction chunk (992 = 8*124)
NCH = D_IN // DCH   # 8
W72 = NB * H1       # 72 layer-1 feature rows
W96 = 96            # silu replica base (32-aligned)
WTOT = W96 + H1     # 102 rows: 0:72 basis, 96:102 silu replicas


# --------------------------------------------------------------------------- #
# Custom DVE ops (registered into concourse.dve_ops.OPS at import)
# --------------------------------------------------------------------------- #
def _register_dve_ops():
    from concourse.dve_spec import (
        Spec, Src0, Src1, C0, C1, sq, relu, lower, _has_src1,
    )
    from concourse.dve_uop import DveOpSpec
    from concourse import dve_ops
    from concourse.dve_ops import DveOp

    def _mk(name, spec):
        if any(o.name == name for o in dve_ops.OPS):
            return next(o for o in dve_ops.OPS if o.name == name)
        row = max(dve_ops._SUB_OPCODE_FOR_NAME.values()) + 1
        assert row < 0x20
        shas = {}
        for ver in ("v3", "v4"):
            try:
                s = DveOpSpec(name=name, opcode=row, uops=lower(spec, ver=ver),
                              rd1_en=_has_src1(spec))
                shas[ver] = s.sha(ver)
            except Exception:
                pass
        op = DveOp(name, spec, subdim=False, uops_sha=shas)
        dve_ops.OPS.append(op)
        dve_ops._SUB_OPCODE_FOR_NAME[name] = row
        dve_ops.CUSTOM_DVE_SPECS[name] = spec
        return op

    def _dve_relu(x):
        return np.maximum(np.nan_to_num(x, nan=0.0, posinf=np.inf,
                                        neginf=-np.inf), 0.0)

    # KAN_Q1: out = relu(s0 - in0)^4 * s1
    q1 = Spec(
        body=sq(sq(relu(C0 - Src0))) * C1,
        reference=lambda in0, in1, s0, s1, imm2:
            _dve_relu(np.asarray(s0, np.float32)
                      - in0.astype(np.float32)) ** 4 * s1,
    )
    # KAN_Q2: out = relu(s0 - in0)^4 * s1 + in1
    q2 = Spec(
        body=sq(sq(relu(C0 - Src0))) * C1 + Src1,
        reference=lambda in0, in1, s0, s1, imm2:
            _dve_relu(np.asarray(s0, np.float32)
                      - in0.astype(np.float32)) ** 4 * s1
            + in1.astype(np.float32),
    )
    return _mk("KAN_Q1_ANT", q1), _mk("KAN_Q2_ANT", q2)


# --------------------------------------------------------------------------- #
# Host-side weight preparation (tiny tensors; fp64 math)
# --------------------------------------------------------------------------- #
def _f16(x):
    return x.astype(np.float16)


def prepare_weights(mlp_w, mlp_b, coef0, wb0, ws0, b0, coef1, wb1, ws1, b1):
    w = {}
    # mm0: Wrep[(d), (g,i)] = -4*mlp_w[d,i], fp16 hi/lo, chunked [8][124,128]
    Weff = (-4.0 * mlp_w.astype(np.float64)).astype(np.float32)   # [992, 32]
    Wrep = np.tile(Weff, (1, 4))                                   # [992, 128]
    Wh = _f16(Wrep)
    Wl = _f16(Wrep - Wh.astype(np.float32))
    w["mm0_wh"] = Wh.reshape(NCH, DCH, 128).copy()
    w["mm0_wl"] = Wl.reshape(NCH, DCH, 128).copy()

    # u0 bias vectors [3][128]: bias[(g,i)] = b - 5.5 - 4*mlp_b[i], b = 4*uc+g
    bias0 = np.zeros((3, 128), np.float32)
    for uc in range(3):
        for g in range(4):
            b = 4 * uc + g
            bias0[uc, g * 32:(g + 1) * 32] = b - 5.5 - 4.0 * mlp_b
    w["u0_bias"] = bias0
    w["silu0_bias"] = mlp_b.astype(np.float32).copy()              # [32]

    # layer-0 coefficients: C0eff[(g,i), col] = -4*coef0[i,i2,b]*ws0[i,i2]
    # cols 0:72 = (rep=b, i2); cols 72:78 = extra replica (any b) for silu1 rows
    cw = coef0.astype(np.float64) * ws0.astype(np.float64)[:, :, None]  # [32,6,12]
    C0eff = np.zeros((3, 128, WTOT), np.float64)
    for uc in range(3):
        for g in range(4):
            b = 4 * uc + g
            for i in range(H):
                col = -4.0 * cw[i, :, b]                  # [6] over i2
                C0eff[uc, g * 32 + i, 0:W72] = np.tile(col, NB)
                C0eff[uc, g * 32 + i, W96:WTOT] = col
    w["c0_eff"] = C0eff.astype(np.float32)

    # wb0eff [(i), col] = -4*wb0[i,i2] replicated
    wb0eff = np.zeros((H, WTOT), np.float64)
    wb0eff[:, 0:W72] = np.tile(-4.0 * wb0.astype(np.float64), (1, NB))
    wb0eff[:, W96:WTOT] = -4.0 * wb0.astype(np.float64)
    w["wb0_eff"] = wb0eff.astype(np.float32)

    # u1 = Abs(P1 + (b - 5.5 - 4*b0[i2])) on rows (b,i2) of P1
    bias1 = np.zeros((W72,), np.float32)
    for b in range(NB):
        for i2 in range(H1):
            bias1[b * H1 + i2] = b - 5.5 - 4.0 * b0[i2]
    w["u1_bias"] = bias1
    w["silu1_bias"] = b0.astype(np.float32).copy()                 # [6]

    # merged layer-1 lhsT [78]: rows (b,i2) -> coef1[i2,0,b]*ws1[i2,0];
    # rows 72+i2 -> wb1[i2,0]
    l1 = np.zeros((WTOT,), np.float64)
    cw1 = coef1.astype(np.float64)[:, 0, :] * ws1.astype(np.float64)[:, 0:1]
    for b in range(NB):
        for i2 in range(H1):
            l1[b * H1 + i2] = cw1[i2, b]
    l1[W96:WTOT] = wb1.astype(np.float64)[:, 0]
    w["l1_lhs"] = l1.astype(np.float32)
    w["b1"] = float(np.asarray(b1).reshape(-1)[0])

    # ---- pack into two constant tensors (single DMA each) ----
    cf = np.zeros((128, 416), np.float32)
    cf[:, 0:3] = w["u0_bias"].T                      # [128,3]
    cf[0:H, 3] = w["silu0_bias"]
    for uc in range(3):
        cf[:, 4 + uc * WTOT: 4 + (uc + 1) * WTOT] = w["c0_eff"][uc]
    cf[0:H, 310:310 + WTOT] = w["wb0_eff"]
    cf[0:W72, 412] = w["u1_bias"]
    cf[0:H1, 413] = w["silu1_bias"]
    cf[0:WTOT, 414] = w["l1_lhs"]
    cf[0, 415] = w["b1"]
    w["const_f32"] = cf
    ch = np.zeros((DCH, 2 * NCH * 128), np.float16)
    ch[:, 0:NCH * 128] = w["mm0_wh"].transpose(1, 0, 2).reshape(DCH, -1)
    ch[:, NCH * 128:] = w["mm0_wl"].transpose(1, 0, 2).reshape(DCH, -1)
    w["const_f16"] = ch
    return w


# --------------------------------------------------------------------------- #
# Bass kernel builder
# --------------------------------------------------------------------------- #
def build_kernel(nblocks=BLOCKS, use_silu=True, repeat=1):
    from contextlib import ExitStack
    import concourse.bass as bass
    import concourse.bacc as bacc
    import concourse.tile as tile
    import concourse.mybir as mybir

    Q1, Q2 = _register_dve_ops()

    f32 = mybir.dt.float32
    f16 = mybir.dt.float16
    AF = mybir.ActivationFunctionType
    npc = nblocks * NBLK

    nc = bacc.Bacc()
    xh_d = nc.dram_tensor("xh", [D_IN, npc], f16, kind="ExternalInput")
    xl_d = nc.dram_tensor("xl", [D_IN, npc], f16, kind="ExternalInput")
    cf_d = nc.dram_tensor("const_f32", [128, 416], f32, kind="ExternalInput")
    ch_d = nc.dram_tensor("const_f16", [DCH, 2 * NCH * 128], f16,
                          kind="ExternalInput")
    out_d = nc.dram_tensor("out", [npc], f32, kind="ExternalOutput")

    with ExitStack() as ctx:
        tc = ctx.enter_context(tile.TileContext(nc))
        wpool = ctx.enter_context(tc.tile_pool(name="weights", bufs=1))
        xpool = ctx.enter_context(tc.tile_pool(name="x", bufs=3))
        upool = ctx.enter_context(tc.tile_pool(name="u", bufs=2))
        bpool = ctx.enter_context(tc.tile_pool(name="basis", bufs=2))
        spool = ctx.enter_context(tc.tile_pool(name="small", bufs=2))
        opool = ctx.enter_context(tc.tile_pool(name="outs", bufs=3))
        ph = ctx.enter_context(tc.tile_pool(name="ps_h", bufs=2, space="PSUM"))
        pp1 = ctx.enter_context(tc.tile_pool(name="ps_p1", bufs=2, space="PSUM"))
        po = ctx.enter_context(tc.tile_pool(name="ps_o", bufs=2, space="PSUM"))

        # ---- constants (two DMAs) ----
        cf_t = wpool.tile([128, 416], f32)
        nc.sync.dma_start(cf_t[:], cf_d[:, :])
        ch_t = wpool.tile([DCH, 2 * NCH * 128], f16)
        nc.sync.dma_start(ch_t[:], ch_d[:, :])
        wh_t = ch_t[:, 0:NCH * 128]
        wl_t = ch_t[:, NCH * 128:2 * NCH * 128]
        u0b_t = cf_t[:, 0:3]
        s0b_t = cf_t[0:H, 3:4]
        c0_t = cf_t[:, 4:4 + 3 * WTOT]
        wb0_t = cf_t[0:H, 310:310 + WTOT]
        u1b_t = cf_t[0:W72, 412:413]
        s1b_t = cf_t[0:H1, 413:414]
        l1_t = cf_t[0:WTOT, 414:415]
        b1_t = cf_t[0:1, 415:416]

        SH = [2.5, 1.5, 0.5]
        CW = [1.0 / 24.0, -5.0 / 24.0, 10.0 / 24.0]

        for blk in range(nblocks * repeat):
            ns = (blk % nblocks) * NBLK
            xh_t = xpool.tile([DCH, NCH * NBLK], f16, tag="xh")
            nc.sync.dma_start(
                xh_t[:].rearrange("p (c n) -> p c n", c=NCH),
                xh_d[:, ns:ns + NBLK].rearrange("(c p) n -> p c n", p=DCH))
            xl_t = xpool.tile([DCH, NCH * NBLK], f16, tag="xl")
            nc.sync.dma_start(
                xl_t[:].rearrange("p (c n) -> p c n", c=NCH),
                xl_d[:, ns:ns + NBLK].rearrange("(c p) n -> p c n", p=DCH))

            # ---- mm0 -> hidden4 = -4*(x@W) replicated, PSUM [128, 512]
            hid = ph.tile([128, NBLK], f32, tag="hid")
            for c in range(NCH):
                wh_c = wh_t[:, c * 128:(c + 1) * 128]
                wl_c = wl_t[:, c * 128:(c + 1) * 128]
                xh_c = xh_t[:, c * NBLK:(c + 1) * NBLK]
                xl_c = xl_t[:, c * NBLK:(c + 1) * NBLK]
                nc.tensor.matmul(hid[:], wh_c, xh_c, start=(c == 0), stop=False)
                nc.tensor.matmul(hid[:], wl_c, xh_c, start=False, stop=False)
                nc.tensor.matmul(hid[:], wh_c, xl_c, start=False,
                                 stop=(c == NCH - 1))

            # ---- layer-0 basis (3 chunks) + silu0
            p1 = pp1.tile([WTOT, NBLK], f32, tag="p1")
            b_tiles = []
            for uc in range(3):
                u_t = upool.tile([128, NBLK], f32, tag=f"u{uc}")
                nc.scalar.activation(u_t[:], hid[:], AF.Abs,
                                     bias=u0b_t[:, uc:uc + 1], scale=1.0)
                a1 = bpool.tile([128, NBLK], f32, tag=f"a1_{uc}")
                nc.vector._custom_dve(Q1, out=a1[:], in0=u_t[:],
                                      s0=SH[0], s1=CW[0])
                a2 = bpool.tile([128, NBLK], f32, tag=f"a2_{uc}")
                nc.vector._custom_dve(Q2, out=a2[:], in0=u_t[:], in1=a1[:],
                                      s0=SH[1], s1=CW[1])
                bb = bpool.tile([128, NBLK], f32, tag=f"bb_{uc}")
                nc.vector._custom_dve(Q2, out=bb[:], in0=u_t[:], in1=a2[:],
                                      s0=SH[2], s1=CW[2])
                b_tiles.append(bb)

            sl0 = spool.tile([H, NBLK], f32, tag="sl0")
            nc.scalar.activation(sl0[:], hid[0:H, :],
                                 AF.Silu if use_silu else AF.Sigmoid,
                                 bias=s0b_t, scale=-0.25)

            # ---- P1 = -4*(spl0 + base0), rows 0:72 (b,i2) + 72:78 replicas
            for uc in range(3):
                nc.tensor.matmul(p1[:], c0_t[:, uc * WTOT:(uc + 1) * WTOT],
                                 b_tiles[uc][:], start=(uc == 0), stop=False)
            nc.tensor.matmul(p1[:], wb0_t, sl0[:], start=False, stop=True)

            # ---- layer-1 basis + silu1 (written into rows 72:78 of b78)
            u1_t = upool.tile([W72, NBLK], f32, tag="u1")
            nc.scalar.activation(u1_t[:], p1[0:W72, :], AF.Abs,
                                 bias=u1b_t, scale=1.0)
            b78 = bpool.tile([WTOT, NBLK], f32, tag="b78")
            nc.gpsimd.memset(b78[64:W96, :], 0.0)
            a1b = bpool.tile([W72, NBLK], f32, tag="a1b")
            nc.vector._custom_dve(Q1, out=a1b[:], in0=u1_t[:],
                                  s0=SH[0], s1=CW[0])
            a2b = bpool.tile([W72, NBLK], f32, tag="a2b")
            nc.vector._custom_dve(Q2, out=a2b[:], in0=u1_t[:], in1=a1b[:],
                                  s0=SH[1], s1=CW[1])
            nc.vector._custom_dve(Q2, out=b78[0:W72, :], in0=u1_t[:],
                                  in1=a2b[:], s0=SH[2], s1=CW[2])
            nc.scalar.activation(b78[W96:WTOT, :], p1[W96:WTOT, :],
                                 AF.Silu if use_silu else AF.Sigmoid,
                                 bias=s1b_t, scale=-0.25)

            oo = po.tile([1, NBLK], f32, tag="oo")
            nc.tensor.matmul(oo[:], l1_t, b78[:], start=True, stop=True)

            ot = opool.tile([1, NBLK], f32, tag="ot")
            nc.scalar.activation(ot[:], oo[:], AF.Identity,
                                 bias=b1_t, scale=1.0)
            nc.sync.dma_start(out_d[None, ns:ns + NBLK], ot[:])

    nc.compile()
    return nc


# --------------------------------------------------------------------------- #
# Host orchestration
# --------------------------------------------------------------------------- #
def _prep_x(node_rep, npc):
    """Per-core (xh, xl) fp16 arrays of shape [992, npc], transposed."""
    n = node_rep.shape[0]
    xs = []
    for c in range(NCORES):
        lo = min(c * npc, n)
        hi = min((c + 1) * npc, n)
        blk = node_rep[lo:hi]
        if blk.shape[0] < npc:
            blk = np.concatenate(
                [blk, np.zeros((npc - blk.shape[0], D_IN), np.float32)], axis=0)
        xh = blk.astype(np.float16)
        xl = (blk - xh.astype(np.float32)).astype(np.float16)
        xs.append((np.ascontiguousarray(xh.T), np.ascontiguousarray(xl.T)))
    return xs


def make_in_maps(node_rep, mlp_w, mlp_b, coef0, wb0, ws0, b0, coef1, wb1, ws1,
                 b1, npc=NPC):
    w = prepare_weights(np.asarray(mlp_w), np.asarray(mlp_b), np.asarray(coef0),
                        np.asarray(wb0), np.asarray(ws0), np.asarray(b0),
                        np.asarray(coef1), np.asarray(wb1), np.asarray(ws1),
                        np.asarray(b1))
    xs = _prep_x(np.asarray(node_rep, np.float32), npc)
    base = {"const_f32": w["const_f32"], "const_f16": w["const_f16"]}
    return [dict(base, xh=xs[c][0], xl=xs[c][1]) for c in range(NCORES)]


def kernel(node_rep, mlp_w, mlp_b, coef0, wb0, ws0, b0, coef1, wb1, ws1, b1):
    from concourse import bass_utils

    node_rep = np.asarray(node_rep, np.float32)
    n_rows = node_rep.shape[0]
    in_maps = make_in_maps(node_rep, mlp_w, mlp_b, coef0, wb0, ws0, b0,
                           coef1, wb1, ws1, b1)
    nc = build_kernel()
    res = bass_utils.run_bass_kernel_spmd(nc, in_maps, list(range(NCORES)))
    outs = [np.asarray(res.results[c]["out"]).reshape(-1)
            for c in range(NCORES)]
    full = np.concatenate(outs)[:n_rows].astype(np.float32)
    return full[:, None]



# revision 17
# speedup vs baseline: 55.4940x; 55.4940x over previous
"""Trainium2 Bass kernel for nn_F_local_72335839200042.

MLP (300000x992 @ 992x32) -> KAN layer (32->6, G=8 K=4 B-splines) -> KAN layer (6->1).

Strategy (pure data parallel over 8 NeuronCores, N sharded):
  - Host: pad N to 8*74*512, shard, cast x to fp16 (single pass: rel err
    ~9e-4 on the output, 20x under the 2e-2 gate), pre-tile to
    [74 blocks][124, 8*512] so each block's DMA is one contiguous
    8KB-per-partition transfer.
  - hidden4 = -4*(x @ mlp_w) replicated 4x across 128 partitions via 8 fp16
    matmuls accumulating in PSUM.
  - B-spline basis: B_b(h) = (a^4 - 5 b^4 + 10 c^4)/24 with atoms
    ((shift - u)_+)^4, u = |4h + 5.5 - b| via ACT Abs (per-partition bias,
    fp16 out); atoms+combine via 3 chained custom DVE ops over the
    concatenated [128, 3*512] fp16 tile (2-byte SBUF operands enable the
    DVE fast modes).
  - Layer contractions on PE in fp16 (1 cyc/row); silu via ACT Swish (fp16).
  - Layer-1 basis on 72 = 12 basis x 6 hidden rows; silu replicas ride the
    same PE pass in rows 96:102; final dot = two chained matmuls (rows 0:72
    and 96:102), +b1 applied on host; output DMA'd straight from PSUM.
"""

import sys

for _p in ("/opt/trn_rl_repo",):
    if _p not in sys.path:
        sys.path.insert(0, _p)

import numpy as np

G = 8
K = 4
NB = G + K          # 12 basis fns per input dim
NBLK = 512          # rows per block (one PSUM bank of fp32)
BLOCKS = 74         # blocks per core
NPC = NBLK * BLOCKS  # rows per core (padded)
NCORES = 8
D_IN = 992
H = 32
H1 = 6
DCH = 124           # contraction chunk (992 = 8*124)
NCH = D_IN // DCH   # 8
W72 = NB * H1       # 72 layer-1 feature rows
W96 = 96            # silu replica base (32-aligned)
WTOT = W96 + H1     # 102 rows: 0:72 basis, 96:102 silu replicas

GRP = 8                    # output blocks batched per PSUM tile / DMA

# cf16 column layout
CW0 = NCH * 128            # 1024: mm0 weight chunks (rows 0:124)
CC0 = CW0                  # 3 * 102 c0_eff chunks
CWB = CC0 + 3 * WTOT       # wb0_eff [32, 102]
CL1 = CWB + WTOT           # GRP slabs of [102, GRP]: slab j has l1 in col j
C16 = CL1 + GRP * GRP      # 1496 total cols


# --------------------------------------------------------------------------- #
# Custom DVE ops (registered into concourse.dve_ops.OPS at import)
# --------------------------------------------------------------------------- #
def _register_dve_ops():
    from concourse.dve_spec import (
        Spec, Src0, Src1, C0, C1, sq, relu, lower, _has_src1,
    )
    from concourse.dve_uop import DveOpSpec
    from concourse import dve_ops
    from concourse.dve_ops import DveOp

    def _mk(name, spec):
        if any(o.name == name for o in dve_ops.OPS):
            return next(o for o in dve_ops.OPS if o.name == name)
        row = max(dve_ops._SUB_OPCODE_FOR_NAME.values()) + 1
        assert row < 0x20
        shas = {}
        for ver in ("v3", "v4"):
            try:
                s = DveOpSpec(name=name, opcode=row, uops=lower(spec, ver=ver),
                              rd1_en=_has_src1(spec))
                shas[ver] = s.sha(ver)
            except Exception:
                pass
        op = DveOp(name, spec, subdim=False, uops_sha=shas)
        dve_ops.OPS.append(op)
        dve_ops._SUB_OPCODE_FOR_NAME[name] = row
        dve_ops.CUSTOM_DVE_SPECS[name] = spec
        return op

    def _dve_relu(x):
        return np.maximum(np.nan_to_num(x, nan=0.0, posinf=np.inf,
                                        neginf=-np.inf), 0.0)

    # KAN_Q1: out = relu(s0 - in0)^4 * s1
    q1 = Spec(
        body=sq(sq(relu(C0 - Src0))) * C1,
        reference=lambda in0, in1, s0, s1, imm2:
            _dve_relu(np.asarray(s0, np.float32)
                      - in0.astype(np.float32)) ** 4 * s1,
    )
    # KAN_Q2: out = relu(s0 - in0)^4 * s1 + in1
    q2 = Spec(
        body=sq(sq(relu(C0 - Src0))) * C1 + Src1,
        reference=lambda in0, in1, s0, s1, imm2:
            _dve_relu(np.asarray(s0, np.float32)
                      - in0.astype(np.float32)) ** 4 * s1
            + in1.astype(np.float32),
    )
    return _mk("KAN_Q1_ANT", q1), _mk("KAN_Q2_ANT", q2)


# --------------------------------------------------------------------------- #
# Host-side weight preparation (tiny tensors; fp64 math)
# --------------------------------------------------------------------------- #
def _f16(x):
    return x.astype(np.float16)


def prepare_weights(mlp_w, mlp_b, coef0, wb0, ws0, b0, coef1, wb1, ws1, b1):
    w = {}
    # mm0: Wrep[(d), (g,i)] = -4*mlp_w[d,i] replicated 4x, fp16, chunks [8][124,128]
    Weff = (-4.0 * mlp_w.astype(np.float64))                       # [992, 32]
    Wrep = np.tile(Weff, (1, 4))                                   # [992, 128]
    wchunks = _f16(Wrep).reshape(NCH, DCH, 128)                    # [8,124,128]

    # cf32 [128, 6]: ACT bias columns
    cf32 = np.zeros((128, 6), np.float32)
    # cols 0..2: u0 bias per chunk uc: bias[(g,i)] = b - 5.5 - 4*mlp_b[i]
    for uc in range(3):
        for g in range(4):
            b = 4 * uc + g
            cf32[g * 32:(g + 1) * 32, uc] = b - 5.5 - 4.0 * mlp_b
    cf32[0:H, 3] = mlp_b                                           # silu0 bias
    # col 4: u1 bias on rows (b,i2): b - 5.5 - 4*b0[i2]
    for b in range(NB):
        for i2 in range(H1):
            cf32[b * H1 + i2, 4] = b - 5.5 - 4.0 * b0[i2]
    cf32[0:H1, 5] = b0                                             # silu1 bias
    w["const_f32"] = cf32

    # layer-0 coefficients: C0eff[(g,i), col] = -4*coef0[i,i2,b]*ws0[i,i2]
    # cols 0:72 = (rep=b, i2); cols 96:102 = extra replica for silu1 rows
    cw = coef0.astype(np.float64) * ws0.astype(np.float64)[:, :, None]  # [32,6,12]
    C0eff = np.zeros((3, 128, WTOT), np.float64)
    for uc in range(3):
        for g in range(4):
            b = 4 * uc + g
            for i in range(H):
                col = -4.0 * cw[i, :, b]                  # [6] over i2
                C0eff[uc, g * 32 + i, 0:W72] = np.tile(col, NB)
                C0eff[uc, g * 32 + i, W96:WTOT] = col

    # wb0eff [(i), col] = -4*wb0[i,i2] replicated
    wb0eff = np.zeros((H, WTOT), np.float64)
    wb0eff[:, 0:W72] = np.tile(-4.0 * wb0.astype(np.float64), (1, NB))
    wb0eff[:, W96:WTOT] = -4.0 * wb0.astype(np.float64)

    # merged layer-1 lhsT [102]: rows (b,i2) -> coef1[i2,0,b]*ws1[i2,0];
    # rows 96:102 -> wb1[i2,0]
    l1 = np.zeros((WTOT,), np.float64)
    cw1 = coef1.astype(np.float64)[:, 0, :] * ws1.astype(np.float64)[:, 0:1]
    for b in range(NB):
        for i2 in range(H1):
            l1[b * H1 + i2] = cw1[i2, b]
    l1[W96:WTOT] = wb1.astype(np.float64)[:, 0]
    w["b1"] = float(np.asarray(b1).reshape(-1)[0])

    # ---- pack all fp16 constants into one tensor (single DMA) ----
    cf16 = np.zeros((128, C16), np.float16)
    cf16[0:DCH, 0:CW0] = _f16(wchunks.transpose(1, 0, 2).reshape(DCH, -1))
    for uc in range(3):
        cf16[:, CC0 + uc * WTOT: CC0 + (uc + 1) * WTOT] = _f16(C0eff[uc])
    cf16[0:H, CWB:CWB + WTOT] = _f16(wb0eff)
    for j in range(GRP):
        cf16[0:WTOT, CL1 + j * GRP + j] = _f16(l1)
    w["const_f16"] = cf16
    return w


# --------------------------------------------------------------------------- #
# Bass kernel builder
# --------------------------------------------------------------------------- #
def build_kernel(nblocks=BLOCKS, repeat=1, use_silu=True):
    from contextlib import ExitStack
    import concourse.bass as bass
    import concourse.bacc as bacc
    import concourse.tile as tile
    import concourse.mybir as mybir

    Q1, Q2 = _register_dve_ops()

    f32 = mybir.dt.float32
    f16 = mybir.dt.float16
    AF = mybir.ActivationFunctionType
    npc = nblocks * NBLK
    NB3 = 3 * NBLK

    nc = bacc.Bacc()
    x_d = nc.dram_tensor("xh", [nblocks, DCH, NCH * NBLK], f16,
                         kind="ExternalInput")
    cf_d = nc.dram_tensor("const_f32", [128, 6], f32, kind="ExternalInput")
    ch_d = nc.dram_tensor("const_f16", [128, C16], f16, kind="ExternalInput")
    out_d = nc.dram_tensor("out", [nblocks, NBLK], f32, kind="ExternalOutput")

    with ExitStack() as ctx:
        tc = ctx.enter_context(tile.TileContext(nc))
        wpool = ctx.enter_context(tc.tile_pool(name="weights", bufs=1))
        xpool = ctx.enter_context(tc.tile_pool(name="x", bufs=3))
        upool = ctx.enter_context(tc.tile_pool(name="u", bufs=2))
        bpool = ctx.enter_context(tc.tile_pool(name="basis", bufs=2))
        spool = ctx.enter_context(tc.tile_pool(name="small", bufs=2))
        opool = ctx.enter_context(tc.tile_pool(name="outs", bufs=2))
        ph = ctx.enter_context(tc.tile_pool(name="ps_h", bufs=2, space="PSUM"))
        pp1 = ctx.enter_context(tc.tile_pool(name="ps_p1", bufs=2, space="PSUM"))
        po = ctx.enter_context(tc.tile_pool(name="ps_o", bufs=2, space="PSUM"))

        # ---- constants (two DMAs) ----
        cf_t = wpool.tile([128, 6], f32)
        nc.sync.dma_start(cf_t[:], cf_d[:, :])
        ch_t = wpool.tile([128, C16], f16)
        nc.sync.dma_start(ch_t[:], ch_d[:, :])

        SH = [2.5, 1.5, 0.5]
        CW = [1.0 / 24.0, -5.0 / 24.0, 10.0 / 24.0]

        oo = None
        for blk in range(nblocks * repeat):
            ib = blk % nblocks
            ig, j = ib // GRP, ib % GRP
            gsz = min(GRP, nblocks - ig * GRP)
            xt = xpool.tile([DCH, NCH * NBLK], f16, tag="xh")
            nc.sync.dma_start(xt[:], x_d[ib])

            # ---- mm0 -> hidden4 = -4*(x@W) replicated, PSUM [128, 512]
            hid = ph.tile([128, NBLK], f32, tag="hid")
            for c in range(NCH):
                nc.tensor.matmul(hid[:],
                                 ch_t[0:DCH, c * 128:(c + 1) * 128],
                                 xt[:, c * NBLK:(c + 1) * NBLK],
                                 start=(c == 0), stop=(c == NCH - 1))

            # ---- layer-0 u (3 chunks into one fp16 tile) + basis chain
            u012 = upool.tile([128, NB3], f16, tag="u012")
            for uc in range(3):
                nc.scalar.activation(u012[:, uc * NBLK:(uc + 1) * NBLK],
                                     hid[:], AF.Abs,
                                     bias=cf_t[:, uc:uc + 1], scale=1.0)
            a1 = bpool.tile([128, NB3], f16, tag="a1")
            nc.vector._custom_dve(Q1, out=a1[:], in0=u012[:],
                                  s0=SH[0], s1=CW[0])
            a2 = bpool.tile([128, NB3], f16, tag="a2")
            nc.vector._custom_dve(Q2, out=a2[:], in0=u012[:], in1=a1[:],
                                  s0=SH[1], s1=CW[1])
            bb = bpool.tile([128, NB3], f16, tag="bb")
            nc.vector._custom_dve(Q2, out=bb[:], in0=u012[:], in1=a2[:],
                                  s0=SH[2], s1=CW[2])

            sl0 = spool.tile([H, NBLK], f16, tag="sl0")
            nc.scalar.activation(sl0[:], hid[0:H, :],
                                 AF.Silu if use_silu else AF.Sigmoid,
                                 bias=cf_t[0:H, 3:4], scale=-0.25)

            # ---- P1 = -4*(spl0 + base0): rows 0:72 (b,i2) + 96:102 replicas
            p1 = pp1.tile([WTOT, NBLK], f32, tag="p1")
            for uc in range(3):
                nc.tensor.matmul(p1[:],
                                 ch_t[:, CC0 + uc * WTOT:CC0 + (uc + 1) * WTOT],
                                 bb[:, uc * NBLK:(uc + 1) * NBLK],
                                 start=(uc == 0), stop=False)
            nc.tensor.matmul(p1[:], ch_t[0:H, CWB:CWB + WTOT], sl0[:],
                             start=False, stop=True)

            # ---- layer-1 basis + silu (rows 96:102); gap rows never touched
            u1 = upool.tile([W72, NBLK], f16, tag="u1")
            nc.scalar.activation(u1[:], p1[0:W72, :], AF.Abs,
                                 bias=cf_t[0:W72, 4:5], scale=1.0)
            b1t = bpool.tile([WTOT, NBLK], f16, tag="b1t")
            nc.gpsimd.memset(b1t[64:W96, :], 0.0)
            a1b = bpool.tile([W72, NBLK], f16, tag="a1b")
            nc.vector._custom_dve(Q1, out=a1b[:], in0=u1[:],
                                  s0=SH[0], s1=CW[0])
            a2b = bpool.tile([W72, NBLK], f16, tag="a2b")
            nc.vector._custom_dve(Q2, out=a2b[:], in0=u1[:], in1=a1b[:],
                                  s0=SH[1], s1=CW[1])
            nc.vector._custom_dve(Q2, out=b1t[0:W72, :], in0=u1[:],
                                  in1=a2b[:], s0=SH[2], s1=CW[2])
            nc.scalar.activation(b1t[W96:WTOT, :], p1[W96:WTOT, :],
                                 AF.Silu if use_silu else AF.Sigmoid,
                                 bias=cf_t[0:H1, 5:6], scale=-0.25)

            if j == 0:
                oo = po.tile([GRP, NBLK], f32, tag="oo")
            nc.tensor.matmul(oo[:],
                             ch_t[0:WTOT, CL1 + j * GRP:CL1 + (j + 1) * GRP],
                             b1t[:], start=(j == 0), stop=(j == gsz - 1))
            if j == gsz - 1:
                ot = opool.tile([GRP, NBLK], f32, tag="ot")
                nc.scalar.copy(ot[0:gsz, :], oo[0:gsz, :])
                nc.sync.dma_start(out_d[ig * GRP:ig * GRP + gsz, :],
                                  ot[0:gsz, :])

    nc.compile()
    return nc


# --------------------------------------------------------------------------- #
# Host orchestration
# --------------------------------------------------------------------------- #
def _prep_x(node_rep, npc):
    """Per-core fp16 arrays [nblocks, 124, 8*512], block-tiled for
    contiguous DMA: A[b, p, c*512+n] = x[b*512+n, c*124+p]."""
    n = node_rep.shape[0]
    nblocks = npc // NBLK
    xs = []
    for c in range(NCORES):
        lo = min(c * npc, n)
        hi = min((c + 1) * npc, n)
        blk = node_rep[lo:hi]
        if blk.shape[0] < npc:
            blk = np.concatenate(
                [blk, np.zeros((npc - blk.shape[0], D_IN), np.float32)], axis=0)
        xh = blk.astype(np.float16)
        A = xh.reshape(nblocks, NBLK, NCH, DCH).transpose(0, 3, 2, 1)
        xs.append(np.ascontiguousarray(A).reshape(nblocks, DCH, NCH * NBLK))
    return xs


def make_in_maps(node_rep, mlp_w, mlp_b, coef0, wb0, ws0, b0, coef1, wb1, ws1,
                 b1, npc=NPC):
    w = prepare_weights(np.asarray(mlp_w), np.asarray(mlp_b), np.asarray(coef0),
                        np.asarray(wb0), np.asarray(ws0), np.asarray(b0),
                        np.asarray(coef1), np.asarray(wb1), np.asarray(ws1),
                        np.asarray(b1))
    xs = _prep_x(np.asarray(node_rep, np.float32), npc)
    base = {"const_f32": w["const_f32"], "const_f16": w["const_f16"]}
    return [dict(base, xh=xs[c]) for c in range(NCORES)], w["b1"]


def kernel(node_rep, mlp_w, mlp_b, coef0, wb0, ws0, b0, coef1, wb1, ws1, b1):
    from concourse import bass_utils

    node_rep = np.asarray(node_rep, np.float32)
    n_rows = node_rep.shape[0]
    in_maps, b1_val = make_in_maps(node_rep, mlp_w, mlp_b, coef0, wb0, ws0, b0,
                                   coef1, wb1, ws1, b1)
    nc = build_kernel()
    res = bass_utils.run_bass_kernel_spmd(nc, in_maps, list(range(NCORES)))
    outs = [np.asarray(res.results[c]["out"]).reshape(-1)
            for c in range(NCORES)]
    full = (np.concatenate(outs)[:n_rows] + b1_val).astype(np.float32)
    return full[:, None]
